# revision 28
# baseline (speedup 1.0000x reference)
"""Trainium2 Bass kernel for MeshGenLoss (Chamfer + KL + density-uniformity).

Math:
  d[i,j] = |a_i|^2 + |b_j|^2 - 2 a_i.b_j  via bf16 limb matmuls: every fp32
  scalar splits into 3 exact bf16 limbs; keeping products with p+q <= 2 plus
  3-limb norms gives K=24 contraction rows (~2^-23 dot error, fp32 PSUM).

PE 4-way row packing: K=24 <= 32, so the 128x128 PE array is split into
  four 32-row groups via tile_position=(32g, 0).  The four matmuls of a
  "quad" hold the same weights in their own row group, stream four different
  512-column slices concurrently (measured cadence ~65ns per 512-col matmul
  vs 439ns unpacked; the PE in this environment is locked at 1.2 GHz), and
  write adjacent PSUM slices.  Host packs lhsT/rhs so group g sits on SBUF
  partitions 32g..32g+23.

Evacuation: ScalarE and VectorE are the only PSUM-capable engines and only
  VectorE can min (Pool cannot touch PSUM nor run min, tensor_tensor_reduce
  hard-faults the device, tensor_tensor_scan runs ~2 cycles/element), and at
  most ONE operand per instruction may read PSUM.  So:
   - pt jobs are COPY-ONLY: all four [128,1024] chunks stream to DRAM as
     fp16 (ScalarE copies chunks 0,2; VectorE tensor_scalar_min-copies
     chunks 1,3), alternating between the sync and scalar HWDGE queues.
     The HOST then takes row-mins (Chamfer pred->target) AND column-mins
     (which ARE the target->pred row-mins, combined across cores) from the
     same matrix -- the 8 target->pred matmul jobs are eliminated entirely.
   - pp (self-distance) jobs pair chunks on-chip: ScalarE copies the even
     chunk to fp16, VectorE takes min(odd_chunk_psum, seed) in one wide 1x
     tensor_tensor, and the [128,1024] min-tile DMAs out; the host does the
     final 1024-way min.  The diagonal is masked by a Pool-engine ADD of 6e4
     onto the fp16 seed (Pool supports add on SBUF).  One all-ACT pp job
     rebalances ScalarE vs VectorE (~37us each).
  KL is computed on the host (1024 elements).

Sharding: core c owns rows [512c, 512c+512) of the pred->target and self
  distance matrices for both batches = 16 jobs of [128 rows x 4096 cols].
  For the self matrix the columns are pre-rotated by 512c on the host so the
  masked diagonal always falls in column-tile 0 (keeps the SPMD program
  identical across cores).
"""

import sys

import ml_dtypes
import numpy as np

sys.path.insert(0, "/opt/trn_rl_repo")

B = 2
N = 4096
L = 512
CORES = 8
ROWS = N // CORES  # 512 rows per core
RB = ROWS // 128  # 4 row blocks per core
K = 24
BF16 = ml_dtypes.bfloat16
FP16 = np.float16
BIG = 3.0e38
DIAG = 6.0e4  # > max squared distance (~50), fp16-safe

# limb-product pairs (p from lhs, q from rhs), p+q <= 2
PAIRS = [(0, 0), (0, 1), (1, 0), (1, 1), (0, 2), (2, 0)]


def _limbs3(x):
    """Split float64 array into 3 bf16 limbs capturing ~24 significand bits."""
    h = x.astype(BF16)
    r = x - h.astype(np.float64)
    m = r.astype(BF16)
    r2 = r - m.astype(np.float64)
    lo = r2.astype(BF16)
    return h, m, lo


def _build_lhsT(a):
    """a: [n, 3] float64 row points -> lhsT [24, n] bf16.

    Rows 0..17: (t, (p,q)) -> -2 * limb_p(a[:, t])
    Rows 18..20: limbs of |a|^2     (partner: ones)
    Rows 21..23: ones               (partner: limbs of |b|^2)
    """
    n = a.shape[0]
    asq = (a * a).sum(-1)
    al = _limbs3(a)
    sl = _limbs3(asq)
    out = np.zeros((K, n), dtype=BF16)
    k = 0
    for t in range(3):
        for p, _q in PAIRS:
            out[k] = (-2.0 * al[p][:, t].astype(np.float64)).astype(BF16)
            k += 1
    for p in range(3):
        out[k] = sl[p]
        k += 1
    for _q in range(3):
        out[k] = np.ones(n, dtype=BF16)
        k += 1
    return out


def _build_rhs(b):
    """b: [m, 3] float64 column points -> rhs [24, m] bf16."""
    m = b.shape[0]
    bsq = (b * b).sum(-1)
    bl = _limbs3(b)
    sl = _limbs3(bsq)
    out = np.zeros((K, m), dtype=BF16)
    k = 0
    for t in range(3):
        for _p, q in PAIRS:
            out[k] = bl[q][:, t]
            k += 1
    for _p in range(3):
        out[k] = np.ones(m, dtype=BF16)
        k += 1
    for q in range(3):
        out[k] = sl[q]
        k += 1
    return out


def _pack_groups(x, free_pack):
    """x: [K, F] -> packed [128, F'] with group g on partitions 32g..32g+K-1.

    free_pack: if True, F = 4096 original columns are split so group g holds
    columns 2048u + 512g + j at packed column 512u + j (u = quad index).
    If False (weights), every group holds the same F columns.
    """
    if free_pack:
        F = x.shape[1]
        nq = F // 2048
        out = np.zeros((128, 512 * nq), dtype=x.dtype)
        for g in range(4):
            for u in range(nq):
                out[32 * g:32 * g + K, 512 * u:512 * (u + 1)] = \
                    x[:, 2048 * u + 512 * g:2048 * u + 512 * g + 512]
    else:
        out = np.zeros((128, x.shape[1]), dtype=x.dtype)
        for g in range(4):
            out[32 * g:32 * g + K] = x
    return out


def _build_program():
    import concourse.bacc as bacc
    import concourse.mybir as mybir
    import concourse.tile as tile
    from contextlib import ExitStack

    dt = mybir.dt
    Alu = mybir.AluOpType
    Act = mybir.ActivationFunctionType

    nc = bacc.Bacc("TRN2", target_bir_lowering=False, debug=False)

    d_lhsT_pt = nc.declare_dram_parameter("lhsT_pt", [B, 128, ROWS], dt.bfloat16, isOutput=False)
    d_rhs_t = nc.declare_dram_parameter("rhs_t", [B, 128, N // 4], dt.bfloat16, isOutput=False)
    d_rhs_p = nc.declare_dram_parameter("rhs_p", [B, 128, N // 4], dt.bfloat16, isOutput=False)
    d_diag = nc.declare_dram_parameter("diag", [128, 128], dt.float16, isOutput=False)

    o_pt = nc.declare_dram_parameter("o_pt", [B, RB, 4, 128, 1024], dt.float16, isOutput=True)
    o_pp = nc.declare_dram_parameter("o_pp", [B, RB, 2, 128, 1024], dt.float16, isOutput=True)
    o_ex = nc.declare_dram_parameter("o_ex", [2, 2, 128, 1024], dt.float16, isOutput=True)

    with tile.TileContext(nc) as tc, ExitStack() as ctx:
        consts = ctx.enter_context(tc.tile_pool(name="consts", bufs=1))
        psum = ctx.enter_context(tc.tile_pool(name="psum", bufs=4, space="PSUM"))
        seedp = ctx.enter_context(tc.tile_pool(name="seedp", bufs=10))
        junkp = ctx.enter_context(tc.tile_pool(name="junkp", bufs=8))
        apool = ctx.enter_context(tc.tile_pool(name="acc", bufs=8))

        # ---- resident inputs (DMA'd in job-consumption order over TWO
        # queues: sync + scalar HWDGE -- ScalarE is idle during startup) ----
        lhsT_sb = {}
        rhs_sb = {}
        def load_rhs(dram, b, tag, eng1, eng2):
            t = consts.tile([128, N // 4], dt.bfloat16, tag=tag)
            eng1.dma_start(out=t[:, :512], in_=dram[b, :, :512])
            eng2.dma_start(out=t[:, 512:], in_=dram[b, :, 512:])
            return t

        # job 1 needs lhsT_pt b0 + rhs_t b0: those go first on sync; later
        # inputs ride the scalar HWDGE queue (ScalarE is idle at startup)
        t1_0 = consts.tile([128, ROWS], dt.bfloat16, tag="lpt0")
        nc.sync.dma_start(out=t1_0[:], in_=d_lhsT_pt[0])
        lhsT_sb["pt", 0] = t1_0
        lhsT_sb["pp", 0] = t1_0
        rhs_sb["pt", 0] = load_rhs(d_rhs_t, 0, "rt0", nc.sync, nc.sync)
        t1_1 = consts.tile([128, ROWS], dt.bfloat16, tag="lpt1")
        nc.scalar.dma_start(out=t1_1[:], in_=d_lhsT_pt[1])
        lhsT_sb["pt", 1] = t1_1
        lhsT_sb["pp", 1] = t1_1
        rhs_sb["pt", 1] = load_rhs(d_rhs_t, 1, "rt1", nc.scalar, nc.scalar)
        for b in range(B):
            r2 = load_rhs(d_rhs_p, b, f"rp{b}", nc.scalar, nc.scalar)
            rhs_sb["pp", b] = r2
        diag_sb = consts.tile([128, 128], dt.float16, tag="diag")
        nc.scalar.dma_start(out=diag_sb[:], in_=d_diag[:])

        # ---- 16 distance-matrix jobs ---------------------------------
        # pt jobs are copy-only: all four chunks go to DRAM unpaired, so the
        # host extracts BOTH the pt row-mins AND (as column-mins) the tp
        # row-mins from one matrix -- the 8 tp matmul jobs are gone entirely.
        # ScalarE copies chunks 0,2 and VectorE (tensor_scalar_min) copies
        # chunks 1,3, keeping both engines loaded.  pp jobs pair chunks as
        # before (self-matrix: only row-mins needed).  One all-ACT pp job
        # rebalances the engines.
        a_set = {11}
        jobs = [(b, r, kind) for kind in ("pt", "pp")
                for b in range(B) for r in range(RB)]
        for jidx, (b, r, kind) in enumerate(jobs):
            lhsT = lhsT_sb["pt", b]
            rhs = rhs_sb[kind, b]
            chunks = []
            for u in range(2):  # quad -> two [128,1024] chunks
                ca = psum.tile([128, 1024], dt.float32, tag="ps")
                cb = psum.tile([128, 1024], dt.float32, tag="ps")
                outs = [ca[:, :512], ca[:, 512:], cb[:, :512], cb[:, 512:]]
                for g in range(4):
                    nc.tensor.matmul(
                        outs[g],
                        lhsT[32 * g:32 * g + K, 128 * r:128 * (r + 1)],
                        rhs[32 * g:32 * g + K, 512 * u:512 * (u + 1)],
                        start=True, stop=True,
                        tile_position=(32 * g, 0),
                    )
                chunks.extend([ca, cb])

            if kind == "pt":
                for h in range(4):
                    sh = seedp.tile([128, 1024], dt.float16, tag="sd")
                    if h % 2 == 0:
                        nc.scalar.copy(sh[:], chunks[h][:])
                    else:
                        nc.vector.tensor_scalar_min(sh[:], chunks[h][:], BIG)
                    # alternate output queues: sync + scalar HWDGE in parallel
                    eng = nc.sync if h % 2 == 0 else nc.scalar
                    eng.dma_start(out=o_pt[b, r, h], in_=sh[:])
                continue

            for pi in range(2):  # chunk pairs (0,1) and (2,3)
                ce, co = chunks[2 * pi], chunks[2 * pi + 1]
                sE = seedp.tile([128, 1024], dt.float16, tag="sd")
                nc.scalar.copy(sE[:], ce[:])
                if pi == 0:
                    sl = sE[:, 128 * r:128 * r + 128]
                    nc.gpsimd.tensor_tensor(sl, sl, diag_sb[:], Alu.add)
                if jidx in a_set:
                    sO = seedp.tile([128, 1024], dt.float16, tag="sd")
                    nc.scalar.copy(sO[:], co[:])
                    nc.sync.dma_start(out=o_pp[b, r, pi], in_=sE[:])
                    nc.sync.dma_start(out=o_ex[pi, 0], in_=sO[:])
                else:
                    m = junkp.tile([128, 1024], dt.float16, tag="jk")
                    nc.vector.tensor_tensor(m[:], co[:], sE[:], Alu.min)
                    nc.sync.dma_start(out=o_pp[b, r, pi], in_=m[:])


    nc.compile()
    return nc


def _make_in_maps(pred, target, mu, logvar):
    pred = np.asarray(pred, dtype=np.float32)
    target = np.asarray(target, dtype=np.float32)
    mu = np.asarray(mu, dtype=np.float32)
    logvar = np.asarray(logvar, dtype=np.float32)

    pred64 = pred.astype(np.float64)
    target64 = target.astype(np.float64)

    # Shared (core-independent) operands, packed for 4-way PE row tiling
    rhs_t = np.stack([_pack_groups(_build_rhs(target64[b]), True) for b in range(B)])
    rhs_p_full = np.stack([_build_rhs(pred64[b]) for b in range(B)])  # [B,K,N]
    diag = (np.eye(128) * DIAG).astype(np.float16)

    in_maps = []
    for c in range(CORES):
        rows = slice(ROWS * c, ROWS * (c + 1))
        lhsT_pt = np.stack([_pack_groups(_build_lhsT(pred64[b, rows]), False) for b in range(B)])
        rot = np.roll(rhs_p_full, -ROWS * c, axis=2)
        rhs_p = np.stack([_pack_groups(rot[b], True) for b in range(B)])
        in_maps.append({
            "lhsT_pt": lhsT_pt,
            "rhs_t": rhs_t,
            "rhs_p": rhs_p,
            "diag": diag,
        })
    return in_maps


def kernel(pred, target, mu, logvar):
    from concourse.bass_utils import run_bass_kernel_spmd

    in_maps = _make_in_maps(pred, target, mu, logvar)
    nc = _build_program()
    res = run_bass_kernel_spmd(nc, in_maps, list(range(CORES)))
    results = res.results

    # the all-ACT pp job is jidx 11 -> pp, b=0, r=3 (jobs: 8 pt then 8 pp)
    AJ_B, AJ_R = 0, 3

    nn_pt_parts, tp_partials, nn_pp_parts = [], [], []
    for r_ in results:
        pt_full = np.asarray(r_["o_pt"])  # [B, RB, 4, 128, 1024] fp16
        # chunk h covers original columns [1024h, 1024(h+1))
        rowmin = pt_full.min(axis=-1).min(axis=2)          # [B, RB, 128]
        nn_pt_parts.append(rowmin.astype(np.float32))
        # column-mins over this core's 512 pred rows -> tp partial [B, 4096]
        colmin = pt_full.min(axis=3).min(axis=1)           # [B, 4, 1024]
        tp_partials.append(colmin.reshape(B, N).astype(np.float32))

        m0 = np.asarray(r_["o_pp"]).min(axis=-1).min(axis=2)  # [B, RB, 128]
        ex = np.asarray(r_["o_ex"]).min(axis=-1)[:, 0]        # [2, 128]
        m0[AJ_B, AJ_R] = np.minimum(m0[AJ_B, AJ_R], np.minimum(ex[0], ex[1]))
        nn_pp_parts.append(m0.astype(np.float32))

    nn_pt = np.concatenate([p.reshape(B, ROWS) for p in nn_pt_parts], axis=1)
    nn_tp = np.minimum.reduce(tp_partials)                 # [B, N]
    nn_pp = np.concatenate([p.reshape(B, ROWS) for p in nn_pp_parts], axis=1)

    nn_pt64 = nn_pt.astype(np.float64)
    nn_tp64 = nn_tp.astype(np.float64)
    nn_pp64 = nn_pp.astype(np.float64)

    cd = (nn_pt64.mean(axis=1) + nn_tp64.mean(axis=1)).mean()

    mu64 = np.asarray(mu, dtype=np.float64)
    lv64 = np.asarray(logvar, dtype=np.float64)
    kl = -0.5 * np.mean(1.0 + lv64 - mu64 ** 2 - np.exp(lv64))

    density = np.std(nn_pp64, axis=1, ddof=1).mean()

    total = cd + 0.001 * kl + 0.1 * density

    return (
        np.float32(total),
        np.float32(cd),
        np.float32(kl),
        np.float32(density),
    )


# revision 29
# speedup vs baseline: 1.0979x; 1.0979x over previous
"""Trainium2 Bass kernel for MeshGenLoss (Chamfer + KL + density-uniformity).

Math:
  d[i,j] = |a_i|^2 + |b_j|^2 - 2 a_i.b_j  via bf16 limb matmuls: every fp32
  scalar splits into 3 exact bf16 limbs; keeping products with p+q <= 2 plus
  3-limb norms gives K=24 contraction rows (~2^-23 dot error, fp32 PSUM).

PE 4-way row packing: K=24 <= 32, so the 128x128 PE array is split into
  four 32-row groups via tile_position=(32g, 0).  The four matmuls of a
  "quad" hold the same weights in their own row group, stream four different
  512-column slices concurrently (measured cadence ~65ns per 512-col matmul
  vs 439ns unpacked; the PE in this environment is locked at 1.2 GHz), and
  write adjacent PSUM slices.  Host packs lhsT/rhs so group g sits on SBUF
  partitions 32g..32g+23.

Evacuation: ScalarE and VectorE are the only PSUM-capable engines and only
  VectorE can min (Pool cannot touch PSUM nor run min, tensor_tensor_reduce
  hard-faults the device, tensor_tensor_scan runs ~2 cycles/element), and at
  most ONE operand per instruction may read PSUM.  So:
   - pt jobs are COPY-ONLY: all four [128,1024] chunks stream to DRAM as
     fp16 (ScalarE copies chunks 0,2; VectorE tensor_scalar_min-copies
     chunks 1,3), alternating between the sync and scalar HWDGE queues.
     The HOST then takes row-mins (Chamfer pred->target) AND column-mins
     (which ARE the target->pred row-mins, combined across cores) from the
     same matrix -- the 8 target->pred matmul jobs are eliminated entirely.
   - pp (self-distance) jobs pair chunks on-chip: ScalarE copies the even
     chunk to fp16, VectorE takes min(odd_chunk_psum, seed) in one wide 1x
     tensor_tensor, and the [128,1024] min-tile DMAs out; the host does the
     final 1024-way min.  The diagonal is masked by a Pool-engine ADD of 6e4
     onto the fp16 seed (Pool supports add on SBUF).  One all-ACT pp job
     rebalances ScalarE vs VectorE (~37us each).
  KL is computed on the host (1024 elements).

Sharding: core c owns rows [512c, 512c+512) of the pred->target and self
  distance matrices for both batches = 16 jobs of [128 rows x 4096 cols].
  For the self matrix the columns are pre-rotated by 512c on the host so the
  masked diagonal always falls in column-tile 0 (keeps the SPMD program
  identical across cores).
"""

import sys

import ml_dtypes
import numpy as np

sys.path.insert(0, "/opt/trn_rl_repo")

B = 2
N = 4096
L = 512
CORES = 8
ROWS = N // CORES  # 512 rows per core
RB = ROWS // 128  # 4 row blocks per core
K = 24
BF16 = ml_dtypes.bfloat16
FP16 = np.float16
BIG = 3.0e38
DIAG = 6.0e4  # > max squared distance (~50), fp16-safe

# limb-product pairs (p from lhs, q from rhs), p+q <= 2
PAIRS = [(0, 0), (0, 1), (1, 0), (1, 1), (0, 2), (2, 0)]


def _limbs3(x):
    """Split float64 array into 3 bf16 limbs capturing ~24 significand bits."""
    h = x.astype(BF16)
    r = x - h.astype(np.float64)
    m = r.astype(BF16)
    r2 = r - m.astype(np.float64)
    lo = r2.astype(BF16)
    return h, m, lo


def _build_lhsT(a):
    """a: [n, 3] float64 row points -> lhsT [24, n] bf16.

    Rows 0..17: (t, (p,q)) -> -2 * limb_p(a[:, t])
    Rows 18..20: limbs of |a|^2     (partner: ones)
    Rows 21..23: ones               (partner: limbs of |b|^2)
    """
    n = a.shape[0]
    asq = (a * a).sum(-1)
    al = _limbs3(a)
    sl = _limbs3(asq)
    out = np.zeros((K, n), dtype=BF16)
    k = 0
    for t in range(3):
        for p, _q in PAIRS:
            out[k] = (-2.0 * al[p][:, t].astype(np.float64)).astype(BF16)
            k += 1
    for p in range(3):
        out[k] = sl[p]
        k += 1
    for _q in range(3):
        out[k] = np.ones(n, dtype=BF16)
        k += 1
    return out


def _build_rhs(b):
    """b: [m, 3] float64 column points -> rhs [24, m] bf16."""
    m = b.shape[0]
    bsq = (b * b).sum(-1)
    bl = _limbs3(b)
    sl = _limbs3(bsq)
    out = np.zeros((K, m), dtype=BF16)
    k = 0
    for t in range(3):
        for _p, q in PAIRS:
            out[k] = bl[q][:, t]
            k += 1
    for _p in range(3):
        out[k] = np.ones(m, dtype=BF16)
        k += 1
    for q in range(3):
        out[k] = sl[q]
        k += 1
    return out


def _pack_groups(x, free_pack):
    """x: [K, F] -> packed [128, F'] with group g on partitions 32g..32g+K-1.

    free_pack: if True, F = 4096 original columns are split so group g holds
    columns 2048u + 512g + j at packed column 512u + j (u = quad index).
    If False (weights), every group holds the same F columns.
    """
    if free_pack:
        F = x.shape[1]
        nq = F // 2048
        out = np.zeros((128, 512 * nq), dtype=x.dtype)
        for g in range(4):
            for u in range(nq):
                out[32 * g:32 * g + K, 512 * u:512 * (u + 1)] = \
                    x[:, 2048 * u + 512 * g:2048 * u + 512 * g + 512]
    else:
        out = np.zeros((128, x.shape[1]), dtype=x.dtype)
        for g in range(4):
            out[32 * g:32 * g + K] = x
    return out


def _build_program():
    import concourse.bacc as bacc
    import concourse.mybir as mybir
    import concourse.tile as tile
    from contextlib import ExitStack

    dt = mybir.dt
    Alu = mybir.AluOpType
    Act = mybir.ActivationFunctionType

    nc = bacc.Bacc("TRN2", target_bir_lowering=False, debug=False)

    d_lhsT_pt = nc.declare_dram_parameter("lhsT_pt", [B, 128, ROWS], dt.bfloat16, isOutput=False)
    d_rhs_t = nc.declare_dram_parameter("rhs_t", [B, 128, N // 4], dt.bfloat16, isOutput=False)
    d_rhs_p = nc.declare_dram_parameter("rhs_p", [B, 128, N // 4], dt.bfloat16, isOutput=False)
    o_pt = nc.declare_dram_parameter("o_pt", [B, RB, 4, 128, 1024], dt.float16, isOutput=True)
    o_pp = nc.declare_dram_parameter("o_pp", [B, RB, 2, 128, 1024], dt.float16, isOutput=True)
    o_ppt = nc.declare_dram_parameter("o_ppt", [B, RB, 128, 512], dt.float16, isOutput=True)

    with tile.TileContext(nc) as tc, ExitStack() as ctx:
        consts = ctx.enter_context(tc.tile_pool(name="consts", bufs=1))
        psum = ctx.enter_context(tc.tile_pool(name="psum", bufs=4, space="PSUM"))
        seedp = ctx.enter_context(tc.tile_pool(name="seedp", bufs=10))
        junkp = ctx.enter_context(tc.tile_pool(name="junkp", bufs=8))
        apool = ctx.enter_context(tc.tile_pool(name="acc", bufs=8))

        # ---- resident inputs (DMA'd in job-consumption order over TWO
        # queues: sync + scalar HWDGE -- ScalarE is idle during startup) ----
        lhsT_sb = {}
        rhs_sb = {}
        def load_rhs(dram, b, tag, eng1, eng2):
            t = consts.tile([128, N // 4], dt.bfloat16, tag=tag)
            eng1.dma_start(out=t[:, :512], in_=dram[b, :, :512])
            eng2.dma_start(out=t[:, 512:], in_=dram[b, :, 512:])
            return t

        # job 1 needs lhsT_pt b0 + rhs_t b0: those go first on sync; later
        # inputs ride the scalar HWDGE queue (ScalarE is idle at startup)
        t1_0 = consts.tile([128, ROWS], dt.bfloat16, tag="lpt0")
        nc.sync.dma_start(out=t1_0[:], in_=d_lhsT_pt[0])
        lhsT_sb["pt", 0] = t1_0
        lhsT_sb["pp", 0] = t1_0
        rhs_sb["pt", 0] = load_rhs(d_rhs_t, 0, "rt0", nc.sync, nc.sync)
        t1_1 = consts.tile([128, ROWS], dt.bfloat16, tag="lpt1")
        nc.scalar.dma_start(out=t1_1[:], in_=d_lhsT_pt[1])
        lhsT_sb["pt", 1] = t1_1
        lhsT_sb["pp", 1] = t1_1
        rhs_sb["pt", 1] = load_rhs(d_rhs_t, 1, "rt1", nc.scalar, nc.scalar)
        for b in range(B):
            r2 = load_rhs(d_rhs_p, b, f"rp{b}", nc.scalar, nc.scalar)
            rhs_sb["pp", b] = r2

        # ---- 16 distance-matrix jobs, ALL copy-only ------------------
        # pt: four chunks to DRAM unpaired; the host extracts row-mins
        # (pred->target) AND column-mins (= target->pred row-mins, combined
        # across cores) -- no target->pred matmuls exist.
        # pp: only a wrapped 2560-column band of the symmetric self matrix is
        # computed (one 2048-col packed quad + one plain 512-col matmul);
        # host combines band row-mins and band column-mins and masks the
        # self-diagonal exactly (known position in the raw copies).
        jobs = [(b, r, kind) for kind in ("pt", "pp")
                for b in range(B) for r in range(RB)]
        for jidx, (b, r, kind) in enumerate(jobs):
            lhsT = lhsT_sb["pt", b]
            rhs = rhs_sb[kind, b]
            nu = 2 if kind == "pt" else 1
            chunks = []
            for u in range(nu):  # quad -> two [128,1024] chunks
                ca = psum.tile([128, 1024], dt.float32, tag="ps")
                cb = psum.tile([128, 1024], dt.float32, tag="ps")
                outs = [ca[:, :512], ca[:, 512:], cb[:, :512], cb[:, 512:]]
                for g in range(4):
                    nc.tensor.matmul(
                        outs[g],
                        lhsT[32 * g:32 * g + K, 128 * r:128 * (r + 1)],
                        rhs[32 * g:32 * g + K, 512 * u:512 * (u + 1)],
                        start=True, stop=True,
                        tile_position=(32 * g, 0),
                    )
                chunks.extend([ca, cb])

            if kind == "pt":
                for h in range(4):
                    sh = seedp.tile([128, 1024], dt.float16, tag="sd")
                    if h % 2 == 0:
                        nc.scalar.copy(sh[:], chunks[h][:])
                    else:
                        nc.vector.tensor_scalar_min(sh[:], chunks[h][:], BIG)
                    # alternate output queues: sync + scalar HWDGE in parallel
                    eng = nc.sync if h % 2 == 0 else nc.scalar
                    eng.dma_start(out=o_pt[b, r, h], in_=sh[:])
                continue

            # pp band tail: ONE plain 512-col matmul on group 0
            ctf = psum.tile([128, 1024], dt.float32, tag="ps")
            nc.tensor.matmul(
                ctf[:, :512],
                lhsT[0:K, 128 * r:128 * (r + 1)],
                rhs[0:K, 512:1024],
                start=True, stop=True,
                tile_position=(0, 0),
            )
            for h in range(2):
                sh = seedp.tile([128, 1024], dt.float16, tag="sd")
                if h == 0:
                    nc.scalar.copy(sh[:], chunks[h][:])
                else:
                    nc.vector.tensor_scalar_min(sh[:], chunks[h][:], BIG)
                eng = nc.sync if h == 0 else nc.scalar
                eng.dma_start(out=o_pp[b, r, h], in_=sh[:])
            st = seedp.tile([128, 512], dt.float16, tag="sdt")
            if jidx % 2 == 0:
                nc.scalar.copy(st[:], ctf[:, :512])
                nc.sync.dma_start(out=o_ppt[b, r], in_=st[:])
            else:
                nc.vector.tensor_scalar_min(st[:], ctf[:, :512], BIG)
                nc.scalar.dma_start(out=o_ppt[b, r], in_=st[:])


    nc.compile()
    return nc


def _make_in_maps(pred, target, mu, logvar):
    pred = np.asarray(pred, dtype=np.float32)
    target = np.asarray(target, dtype=np.float32)
    mu = np.asarray(mu, dtype=np.float32)
    logvar = np.asarray(logvar, dtype=np.float32)

    pred64 = pred.astype(np.float64)
    target64 = target.astype(np.float64)

    # Shared (core-independent) operands, packed for 4-way PE row tiling
    rhs_t = np.stack([_pack_groups(_build_rhs(target64[b]), True) for b in range(B)])
    rhs_p_full = np.stack([_build_rhs(pred64[b]) for b in range(B)])  # [B,K,N]

    in_maps = []
    for c in range(CORES):
        rows = slice(ROWS * c, ROWS * (c + 1))
        lhsT_pt = np.stack([_pack_groups(_build_lhsT(pred64[b, rows]), False) for b in range(B)])
        rot = np.roll(rhs_p_full, -ROWS * c, axis=2)
        # band cols 0-2047 as one packed quad; tail cols 2048-2559 replicated
        # into every group (only group 0 is streamed by the tail matmul)
        rhs_p = np.stack([np.concatenate(
            [_pack_groups(rot[b][:, :2048], True),
             _pack_groups(rot[b][:, 2048:2560], False)], axis=1)
            for b in range(B)])
        in_maps.append({
            "lhsT_pt": lhsT_pt,
            "rhs_t": rhs_t,
            "rhs_p": rhs_p,
        })
    return in_maps


def kernel(pred, target, mu, logvar):
    from concourse.bass_utils import run_bass_kernel_spmd

    in_maps = _make_in_maps(pred, target, mu, logvar)
    nc = _build_program()
    res = run_bass_kernel_spmd(nc, in_maps, list(range(CORES)))
    results = res.results

    lanes = np.arange(128)
    nn_pt_parts, tp_partials, pp_row_parts, pp_col_partials = [], [], [], []
    for ci, r_ in enumerate(results):
        pt_full = np.asarray(r_["o_pt"])  # [B, RB, 4, 128, 1024] fp16
        rowmin = pt_full.min(axis=-1).min(axis=2)          # [B, RB, 128]
        nn_pt_parts.append(rowmin.astype(np.float32))
        colmin = pt_full.min(axis=3).min(axis=1)           # [B, 4, 1024]
        tp_partials.append(colmin.reshape(B, N).astype(np.float32))

        # pp band [B, RB, 128, 2560]: chunks 0,1 (cols 0-2047) + tail
        band = np.concatenate(
            [np.asarray(r_["o_pp"]).transpose(0, 1, 3, 2, 4).reshape(B, RB, 128, 2048),
             np.asarray(r_["o_ppt"])], axis=-1).astype(np.float32)
        for rr in range(RB):  # mask self-distance (band col 128rr+i, lane i)
            band[:, rr, lanes, 128 * rr + lanes] = np.float32(DIAG)
        pp_row_parts.append(band.min(axis=-1))             # [B, RB, 128]
        cp = band.min(axis=2).min(axis=1)                  # [B, 2560]
        full = np.full((B, N), np.float32(DIAG))
        idx = (ROWS * ci + np.arange(2560)) % N
        np.minimum.at(full, (slice(None), idx), cp)
        pp_col_partials.append(full)

    nn_pt = np.concatenate([p.reshape(B, ROWS) for p in nn_pt_parts], axis=1)
    nn_tp = np.minimum.reduce(tp_partials)                 # [B, N]
    pp_rows = np.concatenate([p.reshape(B, ROWS) for p in pp_row_parts], axis=1)
    nn_pp = np.minimum(pp_rows, np.minimum.reduce(pp_col_partials))

    nn_pt64 = nn_pt.astype(np.float64)
    nn_tp64 = nn_tp.astype(np.float64)
    nn_pp64 = nn_pp.astype(np.float64)

    cd = (nn_pt64.mean(axis=1) + nn_tp64.mean(axis=1)).mean()

    mu64 = np.asarray(mu, dtype=np.float64)
    lv64 = np.asarray(logvar, dtype=np.float64)
    kl = -0.5 * np.mean(1.0 + lv64 - mu64 ** 2 - np.exp(lv64))

    density = np.std(nn_pp64, axis=1, ddof=1).mean()

    total = cd + 0.001 * kl + 0.1 * density

    return (
        np.float32(total),
        np.float32(cd),
        np.float32(kl),
        np.float32(density),
    )
